# revision 7
# baseline (speedup 1.0000x reference)
"""GroupNorm + 4-head self-attention + output projection, TRN2 Bass kernel.

Sharding: 8 cores = 4 batches x 2 query-halves.  Each core runs GroupNorm and
the full K/V projection for its batch (duplicated across the 2 cores of a
batch, ~5% extra FLOPs) and attention + output projection for its 2048-query
chunk.  The query chunk is rotated to the front of the token axis on the host
(GroupNorm stats / K / V are permutation-invariant along tokens), so all 8
cores run one identical SPMD program and the unshard is pure concatenation.

Device layout (per core).  The kernel is softmax-throughput bound: the exp of
the 4096x2048x4 sim matrix is the largest single cost, so it is split across
two engines per i-chunk:
  heads 0,1: ACT exp (exact, table-based), bf16 out
  heads 2,3: DVE Schraudolph fast-exp -- one tensor_scalar computing
             round(sim * SCALE*log2e*128 + (127-c)*128) into a uint16 tile,
             whose bits reinterpreted as bf16 equal exp(SCALE*sim) within
             +-3%; fp32->uint16 conversion saturates at 0 (underflow -> +0.0)
             and rounds half-even.  The attn@V / denominator matmuls read the
             tile through .bitcast(bf16).
The softmax denominator normalizes per-head, so the fast-exp's systematic
component cancels; end-to-end rel err ~6e-3 (gate 2e-2).

Pipeline per (j, i): 4 sim matmuls (row-tiled 4-up, concurrent), ACT exp pr0
+ DVE fexp pr1 in parallel, then av/dn matmuls from 3 iterations back
(col-tiled 4-up) so the PE FIFO never waits on exp.  The dn matmuls use a
dense all-ones [128,32] stationary so every partition of the dn bank holds
its head's denominator (full PE-tile utilization, and the per-j epilogue
needs no select/broadcast pass: ln reads the psum bank directly).

Per-j epilogue (5 pieces interleaved into the next j's iterations 2-6):
  ln(dn) -> rcb = exp(-ln d) = 1/d on ACT -> ao = oacc*rcb on DVE ->
  per half: bias-prefill outer-product matmul into psum, projection matmul
  accumulated on top, and the y tile DMA'd to HBM straight from psum
  (no separate bias add / staging copy).

Prologue: x is uploaded bf16 (halves the DMA); GroupNorm is computed as
stats only (bn_stats chasing the x DMA chunk by chunk, bn_aggr fp32) and
FOLDED into the projections: q = (Wq diag(alpha)) x + Wq beta, same for k;
the v bias telescopes through softmax (sum_m attn*vb = vb*denominator) into
the output projection bias, so normalized activations are never
materialized.  V is produced DIRECTLY in the attention layout vS[m,o] by
using the x chunk as the stationary matmul operand (out = x_chunk^T @ Wv'),
so no transposes of any kind are needed.
"""

import numpy as np

HEAD = 4
DIM_HEAD = 32
DIM = 256
GROUPS = 32
EPS = 1e-5
SCALE = DIM_HEAD ** -0.5
N = 4096
NQ = 2048
NCORES = 8
P = 128
JW = 512           # query-chunk width per inner tile
NJ = NQ // JW      # 4
NI = N // P        # 32 key chunks

LOG2E = 1.4426950408889634
FE_A = float(SCALE * LOG2E * 128.0)      # fast-exp multiplier (scale folded)
FE_B = float((127.0 - 0.0430) * 128.0)   # fast-exp bias (Schraudolph c)

_cache = {}


def _get_nc():
    if "nc" in _cache:
        return _cache["nc"]
    from contextlib import ExitStack

    import concourse.bass as bass  # noqa: F401
    import concourse.tile as tile
    from concourse import bacc, mybir

    f32 = mybir.dt.float32
    b16 = mybir.dt.bfloat16
    u16 = mybir.dt.uint16
    AF = mybir.ActivationFunctionType
    ALU = mybir.AluOpType

    # Confine Exp/Ln to the one table set that holds both, so the table-load
    # pass never alternates sets (each switch costs ~1.3us of ACT time).
    # Membership-only edit: set order (= act_func_set_id) is preserved.
    import concourse.bacc as bacc_mod
    from concourse.hw_specs import get_activation_tables as _orig_tables

    def _tables_one_exp_ln_set(arch):
        combo = "natural_log_exp_and_others"
        out = {}
        for name, fns in _orig_tables(arch).items():
            if name != combo:
                fns = {f for f in fns
                       if f not in (AF.Exp, AF.Ln, AF.Square)}
            out[name] = fns
        return out

    bacc_mod.get_activation_tables = _tables_one_exp_ln_set

    nc = bacc.Bacc(None, target_bir_lowering=False)
    x_in = nc.declare_dram_parameter("x", [DIM, N], b16, isOutput=False)
    wqkvT = nc.declare_dram_parameter("wqkvT", [DIM, 3 * P], b16, isOutput=False)
    woutT = nc.declare_dram_parameter("woutT", [P, DIM], b16, isOutput=False)
    # small fp32 constants packed into one tensor / one DMA:
    # cols 0-1 gnw(t0,t1), 2-3 gnb, 4-5 bout, 6-21 blk8
    misc = nc.declare_dram_parameter("misc", [P, 22], f32, isOutput=False)
    blk8T = nc.declare_dram_parameter("blk8T", [16, P], f32, isOutput=False)
    y_out = nc.declare_dram_parameter("y", [DIM, NQ], f32, isOutput=True)

    with ExitStack() as ctx:
        tc = ctx.enter_context(tile.TileContext(nc))
        const = ctx.enter_context(tc.tile_pool(name="const", bufs=1))
        persist = ctx.enter_context(tc.tile_pool(name="persist", bufs=1))
        work = ctx.enter_context(tc.tile_pool(name="work", bufs=3))
        attnp = ctx.enter_context(tc.tile_pool(name="attnp", bufs=2))
        # PSUM budget (8 banks): sim ring 3 slots x 2 banks + oacc 1 + dn 1
        psA = ctx.enter_context(tc.tile_pool(name="psA", bufs=3, space="PSUM"))
        psB = ctx.enter_context(tc.tile_pool(name="psB", bufs=1, space="PSUM"))

        # ---- DMA order (one sync queue, issue-rate-bound): x t0 chunks
        # first with the GroupNorm stats chasing each chunk, then the small
        # consts, then x t1 (stats chasing), then the projection weights.
        xb = [persist.tile([P, N], b16, tag=f"xb{t}", name=f"xb{t}")
              for t in range(2)]
        stats = [work.tile([P, 8, 6], f32, tag=f"stats{t}", name=f"stats{t}")
                 for t in range(2)]

        # x upload on BOTH hardware DGE queues: t0 pieces on the sync
        # queue, t1 pieces on the ACT queue (idle during the prologue), so
        # the two tiles stream in parallel; bn_stats chases each piece.
        for ch in range(8):
            nc.sync.dma_start(
                out=xb[0][:, ch * 512:(ch + 1) * 512],
                in_=x_in[0:P, ch * 512:(ch + 1) * 512],
            )
            nc.scalar.dma_start(
                out=xb[1][:, ch * 512:(ch + 1) * 512],
                in_=x_in[P:2 * P, ch * 512:(ch + 1) * 512],
            )
        misc_sb = const.tile([P, 22], f32, tag="misc")
        nc.sync.dma_start(out=misc_sb, in_=misc[:, :])
        gnw_sb = misc_sb[:, 0:2]
        gnb_sb = misc_sb[:, 2:4]
        bout_sb = [misc_sb[:, 4 + t:5 + t] for t in range(2)]
        blk8_sb = misc_sb[:, 6:22]
        blk8T_sb = const.tile([16, P], f32, tag="blk8T")
        nc.sync.dma_start(out=blk8T_sb, in_=blk8T[:, :])
        wqkv_sb = []
        for t in range(2):
            w = const.tile([P, 3 * P], b16, tag=f"wqkv{t}", name=f"wqkv{t}")
            nc.sync.dma_start(out=w, in_=wqkvT[t * P:(t + 1) * P, :])
            wqkv_sb.append(w)
        wout_sb = const.tile([P, DIM], b16, tag="wout")
        nc.sync.dma_start(out=wout_sb, in_=woutT[:, :])
        # stats in piece-arrival order: t0/t1 pieces land in parallel, so
        # interleave t0/t1 on DVE to chase both queues.  The two
        # latest-arriving t1 pieces go to ACT (Square/Copy with accumulate,
        # scales folding the 1/4096 normalization) to shorten the DVE
        # serial stats stream; their raw sums are merged into t1's
        # aggregate below.
        sqscr = work.tile([P, 512], b16, tag="sqscr")
        acc = work.tile([P, 2, 2], f32, tag="acc")  # [:, {s1,s2}, {p6,p7}]
        for ch in range(8):
            nc.vector.bn_stats(
                out=stats[0][:, ch, :],
                in_=xb[0][:, ch * 512:(ch + 1) * 512],
            )
            if ch < 6:
                nc.vector.bn_stats(
                    out=stats[1][:, ch, :],
                    in_=xb[1][:, ch * 512:(ch + 1) * 512],
                )
        for p in range(2):
            pc = 6 + p
            nc.scalar.activation(
                out=sqscr, in_=xb[1][:, pc * 512:(pc + 1) * 512],
                func=AF.Copy, scale=1.0 / 4096.0,
                accum_out=acc[:, 0, p:p + 1],
            )
            nc.scalar.activation(
                out=sqscr, in_=xb[1][:, pc * 512:(pc + 1) * 512],
                func=AF.Square, scale=1.0 / 64.0,
                accum_out=acc[:, 1, p:p + 1],
            )
        ones32 = const.tile([P, 32], b16, tag="ones32")
        nc.vector.memset(ones32, 1.0)
        eps_sb = const.tile([16, 1], f32, tag="eps")
        nc.vector.memset(eps_sb, EPS)

        wqs = [persist.tile([P, 3 * P], b16, tag=f"wqs{t}", name=f"wqs{t}")
               for t in range(2)]
        be16 = [persist.tile([P, 1], b16, tag=f"be16{t}", name=f"be16{t}")
                for t in range(2)]
        qkvb_ps = psA.tile([P, 4], f32, tag="sim")

        # ---------------- GroupNorm ----------------
        # Stats chase the x DMA (emitted above); here: aggregate + the
        # per-group fold chain, t0 first (its stats arrive first).  The
        # per-channel scale (wqs) runs on ACT (Copy with per-partition
        # scale) so DVE can keep streaming t1's bn_stats.
        # Both tiles' chains batched into one set of [*, 2]-strided ops:
        # mv4 cols = (mean0, Ex2_0, mean1, Ex2_1).
        mv4 = work.tile([P, 4], f32, tag="mv4")
        nc.vector.bn_aggr(out=mv4[:, 0:2], in_=stats[0])
        nc.vector.bn_aggr(out=mv4[:, 2:4], in_=stats[1][:, 0:6, :])
        msq = work.tile([P, 2], f32, tag="msq")
        nc.vector.tensor_mul(msq, mv4[:, 0:4:2], mv4[:, 0:4:2])
        nc.vector.tensor_add(mv4[:, 1:4:2], mv4[:, 1:4:2], msq)
        # t1 cols hold (mean, E[x^2]) over pieces 0-5 only: rescale by 6/8
        # and add the ACT-accumulated (sum/4096, sum_sq/4096) of pieces 6,7
        accs = work.tile([P, 2], f32, tag="accs")
        nc.vector.tensor_add(accs, acc[:, :, 0], acc[:, :, 1])
        nc.vector.tensor_scalar(out=mv4[:, 2:4], in0=mv4[:, 2:4],
                                scalar1=0.75, scalar2=None, op0=ALU.mult)
        nc.vector.tensor_add(mv4[:, 2:4], mv4[:, 2:4], accs)
        # per-group (mean, E[x^2]) for both tiles in one matmul
        gst_ps = psB.tile([16, 4], f32, tag="dn", name="gst_ps")
        nc.tensor.matmul(gst_ps, lhsT=blk8_sb, rhs=mv4, start=True, stop=True)
        mmg = work.tile([16, 2], f32, tag="mmg")
        nc.scalar.activation(out=mmg, in_=gst_ps[:, 0:4:2], func=AF.Square)
        varg = work.tile([16, 2], f32, tag="varg")
        nc.vector.tensor_sub(varg, gst_ps[:, 1:4:2], mmg)
        # rstd = exp(-0.5*ln(var+eps)): ln+exp share one ACT table set
        # with the attention exps (no extra ~2.7us table reload)
        sdg = work.tile([16, 2], f32, tag="sdg")
        nc.scalar.activation(
            out=sdg, in_=varg, func=AF.Ln, bias=eps_sb, scale=1.0
        )
        ms4 = work.tile([16, 4], f32, tag="ms4")
        nc.vector.tensor_copy(ms4[:, 0:4:2], gst_ps[:, 0:4:2])
        nc.scalar.activation(
            out=ms4[:, 1:4:2], in_=sdg, func=AF.Exp, scale=-0.5
        )
        # broadcast group (mean, rstd) to the 8 channels of each group
        cb_ps = psB.tile([P, 4], f32, tag="oacc", name="cb_ps")
        nc.tensor.matmul(cb_ps, lhsT=blk8T_sb, rhs=ms4,
                         start=True, stop=True)
        al2 = persist.tile([P, 2], f32, tag="al2")
        nc.vector.tensor_mul(al2, cb_ps[:, 1:4:2], gnw_sb)
        tmpb = work.tile([P, 2], f32, tag="tmpb")
        nc.vector.tensor_mul(tmpb, cb_ps[:, 0:4:2], al2)
        be2 = persist.tile([P, 2], f32, tag="be2")
        nc.vector.tensor_sub(be2, gnb_sb, tmpb)
        albe = [(al2[:, t:t + 1], be2[:, t:t + 1]) for t in range(2)]
        # ---- fold GroupNorm into the projections: q = Wq'(x_bf) + qb,
        # Wq' = Wq diag(alpha), qb = Wq beta (same for k); the V bias
        # telescopes through attention (sum_m attn*vb = vb*denominator)
        # into the output projection bias: bout2 = bout + Wout vb.
        be16_2 = persist.tile([P, 2], b16, tag="be16_2")
        nc.scalar.activation(out=be16_2, in_=be2, func=AF.Copy)
        be16 = [be16_2[:, t:t + 1] for t in range(2)]
        # t0 scale on DVE (bf16 SBUF operands: 4x mode), t1 on ACT - parallel
        nc.vector.tensor_scalar(out=wqs[0], in0=wqkv_sb[0],
                                scalar1=albe[0][0], scalar2=None,
                                op0=ALU.mult)
        nc.scalar.activation(out=wqs[1], in_=wqkv_sb[1], func=AF.Copy,
                             scale=albe[1][0])
        for sel in range(3):
            for t in range(2):
                nc.tensor.matmul(
                    qkvb_ps[:, sel:sel + 1],
                    lhsT=wqkv_sb[t][:, sel * P:(sel + 1) * P],
                    rhs=be16[t], start=(t == 0), stop=(t == 1),
                )
        qb = persist.tile([P, 1], f32, tag="qb")
        nc.vector.tensor_copy(qb, qkvb_ps[:, 0:1])
        kb = persist.tile([P, 1], f32, tag="kb")
        nc.vector.tensor_copy(kb, qkvb_ps[:, 1:2])
        vb16 = persist.tile([P, 1], b16, tag="vb16")
        bout2 = [persist.tile([P, 1], f32, tag=f"bo2{t}", name=f"bo2{t}")
                 for t in range(2)]

        def emit_bout2():
            # deferred off the prologue critical path (first needed by the
            # j=0 epilogue, ~85us in)
            nc.vector.tensor_copy(vb16, qkvb_ps[:, 2:3])
            for t in range(2):
                bo_ps = psA.tile([P, 1], f32, tag="sim", name=f"bo_ps{t}")
                nc.tensor.matmul(bo_ps, lhsT=wout_sb[:, t * P:(t + 1) * P],
                                 rhs=vb16, start=True, stop=True)
                nc.vector.tensor_add(bout2[t], bo_ps, bout_sb[t])

        # ---------------- QKV projections ----------------
        qT = persist.tile([P, NQ], b16, tag="qT")
        kT = persist.tile([P, N], b16, tag="kT")
        vS = persist.tile([P, N], b16, tag="vS")   # vS[p, i*128+o] = v[i*128+p, o]

        def emit_q(jq):
            ps = psA.tile([P, 2, JW], f32, tag="sim")
            for t in range(2):
                nc.tensor.matmul(
                    ps[:, 0, :], lhsT=wqs[t][:, 0:P],
                    rhs=xb[t][:, jq * 512:(jq + 1) * 512],
                    start=(t == 0), stop=(t == 1),
                )
            nc.vector.tensor_scalar(out=qT[:, jq * 512:(jq + 1) * 512],
                                    in0=ps[:, 0, :], scalar1=qb,
                                    scalar2=None, op0=ALU.add)

        def emit_k(jk, splits=(512,)):
            base = jk * 512
            lo = 0
            for hi in splits:
                ps = psA.tile([P, 2, JW], f32, tag="sim")
                for t in range(2):
                    nc.tensor.matmul(
                        ps[:, 0, 0:hi - lo], lhsT=wqs[t][:, P:2 * P],
                        rhs=xb[t][:, base + lo:base + hi],
                        start=(t == 0), stop=(t == 1),
                    )
                nc.scalar.activation(out=kT[:, base + lo:base + hi],
                                     in_=ps[:, 0, 0:hi - lo],
                                     func=AF.Identity, bias=kb, scale=1.0)
                lo = hi

        def emit_vS(ch):
            # one 512-token chunk of v, produced DIRECTLY in the attention
            # layout vS[m, o]: the x chunk is the stationary operand, so
            # out = x_chunk^T @ Wv' = v[m, o] -- no transposes needed.
            ps = psA.tile([P, 2, JW], f32, tag="sim", name="vsps")
            for blk in range(4):
                base = ch * 512 + blk * 128
                for t in range(2):
                    nc.tensor.matmul(
                        ps[:, 0, blk * 128:(blk + 1) * 128],
                        lhsT=xb[t][:, base:base + 128],
                        rhs=wqs[t][:, 2 * P:3 * P],
                        start=(t == 0), stop=(t == 1),
                    )
            if ch % 2 == 0:
                nc.scalar.activation(out=vS[:, ch * 512:(ch + 1) * 512],
                                     in_=ps[:, 0, :], func=AF.Copy)
            else:
                nc.vector.tensor_copy(vS[:, ch * 512:(ch + 1) * 512],
                                      ps[:, 0, :])

        # Produce only what attention j=0 needs up front; the rest (q 1-3,
        # k 1-7, v 4-31) is emitted interleaved into j=0's i-loop so the
        # first exp starts early.
        emit_k(0, splits=(128, 512))
        emit_q(0)

        # ---------------- attention ----------------
        # Per-j epilogue is emitted as 5 pieces interleaved into the first
        # iterations of the NEXT j (overlaps its serial chain with compute
        # and keeps the PE warm across the boundary).
        def make_epilogue(j, oacc, dn):
            def p0():
                # every partition of the dn bank already holds its head's
                # denominator (dense all-ones dn stationary), so ln reads
                # the psum bank directly -- no select/broadcast pass.
                lnd = work.tile([P, JW], f32, tag="lnd")
                nc.scalar.activation(out=lnd, in_=dn, func=AF.Ln)
                return lnd

            def p1(lnd):
                rcb = work.tile([P, JW], f32, tag="rcb")
                nc.scalar.activation(out=rcb, in_=lnd, func=AF.Exp, scale=-1.0)
                return rcb

            def p1b(rcb):
                ao = work.tile([P, JW], b16, tag="ao")
                nc.vector.tensor_mul(ao, oacc, rcb)
                return ao

            def p2(ao, t):
                yps = psA.tile([P, JW], f32, tag="sim")
                nc.tensor.matmul(
                    yps, lhsT=wout_sb[:, t * P:(t + 1) * P], rhs=ao,
                    start=True, stop=True,
                )
                ysb = work.tile([P, JW], f32, tag=f"ysb{t}", name=f"ysb{t}")
                # bias add on ACT (Identity with per-partition bias): keeps
                # the busier DVE free for the fexp stream
                nc.scalar.activation(out=ysb, in_=yps, func=AF.Identity,
                                     bias=bout2[t], scale=1.0)
                nc.sync.dma_start(
                    out=y_out[t * P:(t + 1) * P, j * JW:(j + 1) * JW], in_=ysb
                )

            state = {}

            def run_piece(k):
                if k == 0:
                    state["lnd"] = p0()
                elif k == 1:
                    state["rcb"] = p1(state["lnd"])
                elif k == 2:
                    state["ao"] = p1b(state["rcb"])
                elif k == 3:
                    p2(state["ao"], 0)
                elif k == 4:
                    p2(state["ao"], 1)

            return run_piece

        NPIECE = 5
        AVDELAY = 3
        epilogue = None
        pending = []        # av/dn emission pipeline, carried ACROSS j
        for j in range(NJ):
            oacc = psB.tile([P, JW], f32, tag="oacc")
            dn = psB.tile([P, JW], f32, tag="dn")

            def emit_avdn(i, at0, at1, oacc=oacc, dn=dn):
                rhss = [at0[:, 0, :], at0[:, 1, :],
                        at1[:, 0, :].bitcast(b16), at1[:, 1, :].bitcast(b16)]
                for h in range(HEAD):
                    nc.tensor.matmul(
                        oacc[32 * h:32 * h + 32, :],
                        lhsT=vS[:, i * P + 32 * h:i * P + 32 * h + 32],
                        rhs=rhss[h],
                        start=(i == 0), stop=(i == NI - 1),
                        tile_position=(0, 32 * h),
                        skip_group_check=True,
                    )
                for h in range(HEAD):
                    # dense all-ones stationary: all 32 partitions of each
                    # head's dn block receive the denominator (broadcast
                    # done by the PE for free; full tile utilization).
                    nc.tensor.matmul(
                        dn[32 * h:32 * h + 32, :],
                        lhsT=ones32,
                        rhs=rhss[h],
                        start=(i == 0), stop=(i == NI - 1),
                        tile_position=(0, 32 * h),
                        skip_group_check=True,
                    )

            for i in range(NI):
                if j == 0:
                    if i == 0:
                        emit_vS(0)
                    elif (i + 3) % 4 == 0 and i <= 25:
                        ch = (i + 3) // 4
                        emit_k(ch)
                        emit_vS(ch)
                    elif i == 3:
                        emit_bout2()
                    if i in (2, 4, 6):
                        emit_q(i // 2)
                sims = []
                for pr in range(2):
                    sim = psA.tile([P, 2, JW], f32, tag="sim")
                    for hh in range(2):
                        h = pr * 2 + hh
                        nc.tensor.matmul(
                            sim[:, hh, :],
                            lhsT=kT[32 * h:32 * h + 32, i * P:(i + 1) * P],
                            rhs=qT[32 * h:32 * h + 32, j * JW:(j + 1) * JW],
                            start=True, stop=True,
                            tile_position=(32 * h, 0),
                        )
                    sims.append(sim)
                # heads 0,1: exact exp on ACT; heads 2,3: fast-exp on DVE
                at0 = attnp.tile([P, 2, JW], b16, tag="at0", bufs=5)
                nc.scalar.activation(out=at0, in_=sims[0], func=AF.Exp,
                                     scale=SCALE)
                at1 = attnp.tile([P, 2, JW], u16, tag="at1", bufs=5)
                nc.vector.tensor_scalar(
                    out=at1, in0=sims[1], scalar1=FE_A, scalar2=FE_B,
                    op0=ALU.mult, op1=ALU.add,
                )
                pending.append((emit_avdn, i, at0, at1))
                if len(pending) > AVDELAY:
                    fn, ii, a0, a1 = pending.pop(0)
                    fn(ii, a0, a1)
                if epilogue is not None and 2 <= i < 2 + NPIECE:
                    epilogue(i - 2)
                    if i == 1 + NPIECE:
                        epilogue = None
            epilogue = make_epilogue(j, oacc, dn)
        for fn, ii, a0, a1 in pending:
            fn(ii, a0, a1)
        for k in range(NPIECE):
            epilogue(k)

    nc.finalize()
    _cache["nc"] = nc
    return nc


def _prep_in_maps(x, gn_weight, gn_bias, w_qkv, w_out, b_out):
    import ml_dtypes

    f = np.float32
    bf = ml_dtypes.bfloat16
    x = np.asarray(x, dtype=f).astype(bf)
    wqkvT = np.ascontiguousarray(np.asarray(w_qkv, dtype=f).T.astype(bf))
    woutT = np.ascontiguousarray(np.asarray(w_out, dtype=f).T.astype(bf))
    gnw = np.asarray(gn_weight, dtype=f).reshape(2, P)
    gnb = np.asarray(gn_bias, dtype=f).reshape(2, P)
    bo = np.asarray(b_out, dtype=f).reshape(2, P)
    ar = np.arange(P)
    # misc pack: cols 0-1 gnw(t0,t1), 2-3 gnb, 4-5 unused, 6-21 blk8
    misc = np.zeros((P, 22), f)
    misc[:, 0] = gnw[0]
    misc[:, 1] = gnw[1]
    misc[:, 2] = gnb[0]
    misc[:, 3] = gnb[1]
    misc[:, 4] = bo[0]
    misc[:, 5] = bo[1]
    misc[ar, 6 + ar // 8] = 0.125
    blk8T = np.zeros((16, P), f)
    blk8T[ar // 8, ar] = 1.0
    shared = dict(wqkvT=wqkvT, woutT=woutT, misc=misc, blk8T=blk8T)
    in_maps = []
    for core in range(NCORES):
        b, half = divmod(core, 2)
        xb = x[b].reshape(DIM, N)
        if half == 0:
            xp = np.ascontiguousarray(xb)
        else:
            xp = np.ascontiguousarray(
                np.concatenate([xb[:, NQ:], xb[:, :NQ]], axis=1)
            )
        in_maps.append(dict(x=xp, **shared))
    return in_maps


def _get_executor():
    """Build the sharded jitted executor once (compiles the NEFF once).

    Returns (exec_fn, meta): exec_fn takes a list of 8 per-core input dicts
    and returns the list of 8 per-core output dicts.  Mirrors
    concourse.bass2jax.run_bass_via_pjrt's multi-core path but caches the
    jax.jit so repeated calls don't recompile.
    """
    if "exec" in _cache:
        return _cache["exec"]
    import jax
    import concourse.mybir as mybir
    from jax.sharding import Mesh, PartitionSpec
    from jax.experimental.shard_map import shard_map
    from concourse import bass2jax

    bass2jax.install_neuronx_cc_hook()
    nc = _get_nc()

    partition_name = (
        nc.partition_id_tensor.name if nc.partition_id_tensor else None
    )
    in_names, out_names, out_avals, zero_outs = [], [], [], []
    for alloc in nc.m.functions[0].allocations:
        if not isinstance(alloc, mybir.MemoryLocationSet):
            continue
        name = alloc.memorylocations[0].name
        if alloc.kind == "ExternalInput":
            if name != partition_name:
                in_names.append(name)
        elif alloc.kind == "ExternalOutput":
            shape = tuple(alloc.tensor_shape)
            dtype = mybir.dt.np(alloc.dtype)
            out_names.append(name)
            out_avals.append(jax.core.ShapedArray(shape, dtype))
            zero_outs.append(np.zeros(shape, dtype))
    n_params = len(in_names)
    n_outs = len(out_names)
    all_names = in_names + out_names
    if partition_name is not None:
        all_names = all_names + [partition_name]

    def _body(*args):
        operands = list(args)
        if partition_name is not None:
            operands.append(bass2jax.partition_id_tensor())
        outs = bass2jax._bass_exec_p.bind(
            *operands,
            out_avals=tuple(out_avals),
            in_names=tuple(all_names),
            out_names=tuple(out_names),
            lowering_input_output_aliases=(),
            sim_require_finite=True,
            sim_require_nnan=True,
            nc=nc,
        )
        return tuple(outs)

    devices = jax.devices()[:NCORES]
    mesh = Mesh(np.asarray(devices), ("core",))
    sharded = jax.jit(
        shard_map(
            _body, mesh=mesh,
            in_specs=(PartitionSpec("core"),) * (n_params + n_outs),
            out_specs=(PartitionSpec("core"),) * n_outs,
            check_rep=False,
        ),
        keep_unused=True,
    )
    from jax.sharding import NamedSharding
    sharding = NamedSharding(mesh, PartitionSpec("core"))
    dev_zeros = [
        jax.device_put(
            np.zeros((NCORES * z.shape[0], *z.shape[1:]), z.dtype), sharding
        )
        for z in zero_outs
    ]

    def put_inputs(in_maps):
        return [
            jax.device_put(
                np.concatenate([np.asarray(m[name]) for m in in_maps], axis=0),
                sharding,
            )
            for name in in_names
        ]

    def run_device(device_inputs):
        return sharded(*device_inputs, *dev_zeros)

    def exec_fn(in_maps, device_inputs=None):
        if device_inputs is None:
            device_inputs = put_inputs(in_maps)
        out_arrs = [np.asarray(a) for a in run_device(device_inputs)]
        return [
            {
                name: out_arrs[i].reshape(NCORES, *out_avals[i].shape)[c]
                for i, name in enumerate(out_names)
            }
            for c in range(NCORES)
        ]

    meta = dict(in_names=in_names, out_names=out_names, mesh=mesh,
                sharded=sharded, zero_outs=zero_outs,
                put_inputs=put_inputs, run_device=run_device)
    _cache["exec"] = (exec_fn, meta)
    return _cache["exec"]


def _assemble(results):
    y = np.empty((4, DIM, N), np.float32)
    for core in range(NCORES):
        b, half = divmod(core, 2)
        y[b][:, half * NQ:(half + 1) * NQ] = results[core]["y"]
    return y.reshape(4, DIM, 64, 64)


def _run(inputs, **kw):
    exec_fn, _ = _get_executor()
    in_maps = _prep_in_maps(**inputs)
    results = exec_fn(in_maps)
    return _assemble(results), results


def kernel(x, gn_weight, gn_bias, w_qkv, w_out, b_out):
    out, _ = _run(dict(x=x, gn_weight=gn_weight, gn_bias=gn_bias,
                       w_qkv=w_qkv, w_out=w_out, b_out=b_out))
    return out


# revision 8
# speedup vs baseline: 1.0140x; 1.0140x over previous
"""GroupNorm + 4-head self-attention + output projection, TRN2 Bass kernel.

Sharding: 8 cores = 4 batches x 2 query-halves.  Each core runs GroupNorm and
the full K/V projection for its batch (duplicated across the 2 cores of a
batch, ~5% extra FLOPs) and attention + output projection for its 2048-query
chunk.  The query chunk is rotated to the front of the token axis on the host
(GroupNorm stats / K / V are permutation-invariant along tokens), so all 8
cores run one identical SPMD program and the unshard is pure concatenation.

Device layout (per core).  The kernel is softmax-throughput bound: the exp of
the 4096x2048x4 sim matrix is the largest single cost, so it is split across
two engines per i-chunk:
  heads 0,1: ACT exp (exact, table-based), bf16 out
  heads 2,3: DVE Schraudolph fast-exp -- one tensor_scalar computing
             round(sim * SCALE*log2e*128 + (127-c)*128) into a uint16 tile,
             whose bits reinterpreted as bf16 equal exp(SCALE*sim) within
             +-3%; fp32->uint16 conversion saturates at 0 (underflow -> +0.0)
             and rounds half-even.  The attn@V / denominator matmuls read the
             tile through .bitcast(bf16).
The softmax denominator normalizes per-head, so the fast-exp's systematic
component cancels; end-to-end rel err ~6e-3 (gate 2e-2).

Pipeline per (j, i): 4 sim matmuls (row-tiled 4-up, concurrent), ACT exp pr0
+ DVE fexp pr1 in parallel, then av/dn matmuls from 3 iterations back
(col-tiled 4-up) so the PE FIFO never waits on exp.  The dn matmuls use a
dense all-ones [128,32] stationary so every partition of the dn bank holds
its head's denominator (full PE-tile utilization, and the per-j epilogue
needs no select/broadcast pass: ln reads the psum bank directly).

Per-j epilogue (5 pieces interleaved into the next j's iterations 2-6):
  ln(dn) -> rcb = exp(-ln d) = 1/d on ACT -> ao = oacc*rcb on DVE ->
  per half: bias-prefill outer-product matmul into psum, projection matmul
  accumulated on top, and the y tile DMA'd to HBM straight from psum
  (no separate bias add / staging copy).

Prologue: x is uploaded bf16 (halves the DMA); GroupNorm is computed as
stats only (bn_stats chasing the x DMA chunk by chunk, bn_aggr fp32) and
FOLDED into the projections: q = (Wq diag(alpha)) x + Wq beta, same for k;
the v bias telescopes through softmax (sum_m attn*vb = vb*denominator) into
the output projection bias, so normalized activations are never
materialized.  V is produced DIRECTLY in the attention layout vS[m,o] by
using the x chunk as the stationary matmul operand (out = x_chunk^T @ Wv'),
so no transposes of any kind are needed.
"""

import numpy as np

HEAD = 4
DIM_HEAD = 32
DIM = 256
GROUPS = 32
EPS = 1e-5
SCALE = DIM_HEAD ** -0.5
N = 4096
NQ = 2048
NCORES = 8
P = 128
JW = 512           # query-chunk width per inner tile
NJ = NQ // JW      # 4
NI = N // P        # 32 key chunks

LOG2E = 1.4426950408889634
FE_A = float(SCALE * LOG2E * 128.0)      # fast-exp multiplier (scale folded)
FE_B = float((127.0 - 0.0430) * 128.0)   # fast-exp bias (Schraudolph c)

_cache = {}


def _get_nc():
    if "nc" in _cache:
        return _cache["nc"]
    from contextlib import ExitStack

    import concourse.bass as bass  # noqa: F401
    import concourse.tile as tile
    from concourse import bacc, mybir

    f32 = mybir.dt.float32
    b16 = mybir.dt.bfloat16
    u16 = mybir.dt.uint16
    AF = mybir.ActivationFunctionType
    ALU = mybir.AluOpType

    # Confine Exp/Ln to the one table set that holds both, so the table-load
    # pass never alternates sets (each switch costs ~1.3us of ACT time).
    # Membership-only edit: set order (= act_func_set_id) is preserved.
    import concourse.bacc as bacc_mod
    from concourse.hw_specs import get_activation_tables as _orig_tables

    def _tables_one_exp_ln_set(arch):
        combo = "natural_log_exp_and_others"
        out = {}
        for name, fns in _orig_tables(arch).items():
            if name != combo:
                fns = {f for f in fns
                       if f not in (AF.Exp, AF.Ln, AF.Square,
                                    AF.Copy, AF.Identity)}
            out[name] = fns
        return out

    bacc_mod.get_activation_tables = _tables_one_exp_ln_set

    nc = bacc.Bacc(None, target_bir_lowering=False)
    x_in = nc.declare_dram_parameter("x", [DIM, N], b16, isOutput=False)
    wqkvT = nc.declare_dram_parameter("wqkvT", [DIM, 3 * P], b16, isOutput=False)
    woutT = nc.declare_dram_parameter("woutT", [P, DIM], b16, isOutput=False)
    # small fp32 constants packed into one tensor / one DMA:
    # cols 0-1 gnw(t0,t1), 2-3 gnb, 4-5 bout, 6-21 blk8
    misc = nc.declare_dram_parameter("misc", [P, 22], f32, isOutput=False)
    blk8T = nc.declare_dram_parameter("blk8T", [16, P], f32, isOutput=False)
    y_out = nc.declare_dram_parameter("y", [DIM, NQ], f32, isOutput=True)

    with ExitStack() as ctx:
        tc = ctx.enter_context(tile.TileContext(nc))
        const = ctx.enter_context(tc.tile_pool(name="const", bufs=1))
        persist = ctx.enter_context(tc.tile_pool(name="persist", bufs=1))
        work = ctx.enter_context(tc.tile_pool(name="work", bufs=3))
        attnp = ctx.enter_context(tc.tile_pool(name="attnp", bufs=2))
        # PSUM budget (8 banks): sim ring 3 slots x 2 banks + oacc 1 + dn 1
        psA = ctx.enter_context(tc.tile_pool(name="psA", bufs=3, space="PSUM"))
        psB = ctx.enter_context(tc.tile_pool(name="psB", bufs=1, space="PSUM"))

        # ---- DMA order (one sync queue, issue-rate-bound): x t0 chunks
        # first with the GroupNorm stats chasing each chunk, then the small
        # consts, then x t1 (stats chasing), then the projection weights.
        xb = [persist.tile([P, N], b16, tag=f"xb{t}", name=f"xb{t}")
              for t in range(2)]
        stats = [work.tile([P, 8, 6], f32, tag=f"stats{t}", name=f"stats{t}")
                 for t in range(2)]

        # x upload on BOTH hardware DGE queues: t0 pieces on the sync
        # queue, t1 pieces on the ACT queue (idle during the prologue), so
        # the two tiles stream in parallel; bn_stats chases each piece.
        for ch in range(8):
            nc.sync.dma_start(
                out=xb[0][:, ch * 512:(ch + 1) * 512],
                in_=x_in[0:P, ch * 512:(ch + 1) * 512],
            )
            nc.scalar.dma_start(
                out=xb[1][:, ch * 512:(ch + 1) * 512],
                in_=x_in[P:2 * P, ch * 512:(ch + 1) * 512],
            )
        misc_sb = const.tile([P, 22], f32, tag="misc")
        nc.sync.dma_start(out=misc_sb, in_=misc[:, :])
        gnw_sb = misc_sb[:, 0:2]
        gnb_sb = misc_sb[:, 2:4]
        bout_sb = [misc_sb[:, 4 + t:5 + t] for t in range(2)]
        blk8_sb = misc_sb[:, 6:22]
        blk8T_sb = const.tile([16, P], f32, tag="blk8T")
        nc.sync.dma_start(out=blk8T_sb, in_=blk8T[:, :])
        wqkv_sb = []
        for t in range(2):
            w = const.tile([P, 3 * P], b16, tag=f"wqkv{t}", name=f"wqkv{t}")
            nc.sync.dma_start(out=w, in_=wqkvT[t * P:(t + 1) * P, :])
            wqkv_sb.append(w)
        wout_sb = const.tile([P, DIM], b16, tag="wout")
        nc.sync.dma_start(out=wout_sb, in_=woutT[:, :])
        # stats in piece-arrival order: t0/t1 pieces land in parallel, so
        # interleave t0/t1 on DVE to chase both queues.
        for ch in range(8):
            for t in range(2):
                nc.vector.bn_stats(
                    out=stats[t][:, ch, :],
                    in_=xb[t][:, ch * 512:(ch + 1) * 512],
                )
        ones32 = const.tile([P, 32], b16, tag="ones32")
        nc.vector.memset(ones32, 1.0)
        eps_sb = const.tile([16, 1], f32, tag="eps")
        nc.vector.memset(eps_sb, EPS)

        wqs = [persist.tile([P, 3 * P], b16, tag=f"wqs{t}", name=f"wqs{t}")
               for t in range(2)]
        be16 = [persist.tile([P, 1], b16, tag=f"be16{t}", name=f"be16{t}")
                for t in range(2)]
        qkvb_ps = psA.tile([P, 4], f32, tag="sim")

        # ---------------- GroupNorm ----------------
        # Stats chase the x DMA (emitted above); here: aggregate + the
        # per-group fold chain, t0 first (its stats arrive first).  The
        # per-channel scale (wqs) runs on ACT (Copy with per-partition
        # scale) so DVE can keep streaming t1's bn_stats.
        # Both tiles' chains batched into one set of [*, 2]-strided ops:
        # mv4 cols = (mean0, Ex2_0, mean1, Ex2_1).
        mv4 = work.tile([P, 4], f32, tag="mv4")
        for t in range(2):
            nc.vector.bn_aggr(out=mv4[:, 2 * t:2 * t + 2], in_=stats[t])
        msq = work.tile([P, 2], f32, tag="msq")
        nc.vector.tensor_mul(msq, mv4[:, 0:4:2], mv4[:, 0:4:2])
        nc.vector.tensor_add(mv4[:, 1:4:2], mv4[:, 1:4:2], msq)
        # per-group (mean, E[x^2]) for both tiles in one matmul
        gst_ps = psB.tile([16, 4], f32, tag="dn", name="gst_ps")
        nc.tensor.matmul(gst_ps, lhsT=blk8_sb, rhs=mv4, start=True, stop=True)
        mmg = work.tile([16, 2], f32, tag="mmg")
        nc.scalar.activation(out=mmg, in_=gst_ps[:, 0:4:2], func=AF.Square)
        varg = work.tile([16, 2], f32, tag="varg")
        nc.vector.tensor_sub(varg, gst_ps[:, 1:4:2], mmg)
        # rstd = exp(-0.5*ln(var+eps)): ln+exp share one ACT table set
        # with the attention exps (no extra ~2.7us table reload)
        sdg = work.tile([16, 2], f32, tag="sdg")
        nc.scalar.activation(
            out=sdg, in_=varg, func=AF.Ln, bias=eps_sb, scale=1.0
        )
        ms4 = work.tile([16, 4], f32, tag="ms4")
        nc.vector.tensor_copy(ms4[:, 0:4:2], gst_ps[:, 0:4:2])
        nc.scalar.activation(
            out=ms4[:, 1:4:2], in_=sdg, func=AF.Exp, scale=-0.5
        )
        # broadcast group (mean, rstd) to the 8 channels of each group
        cb_ps = psB.tile([P, 4], f32, tag="oacc", name="cb_ps")
        nc.tensor.matmul(cb_ps, lhsT=blk8T_sb, rhs=ms4,
                         start=True, stop=True)
        al2 = persist.tile([P, 2], f32, tag="al2")
        nc.vector.tensor_mul(al2, cb_ps[:, 1:4:2], gnw_sb)
        tmpb = work.tile([P, 2], f32, tag="tmpb")
        nc.vector.tensor_mul(tmpb, cb_ps[:, 0:4:2], al2)
        be2 = persist.tile([P, 2], f32, tag="be2")
        nc.vector.tensor_sub(be2, gnb_sb, tmpb)
        albe = [(al2[:, t:t + 1], be2[:, t:t + 1]) for t in range(2)]
        # ---- fold GroupNorm into the projections: q = Wq'(x_bf) + qb,
        # Wq' = Wq diag(alpha), qb = Wq beta (same for k); the V bias
        # telescopes through attention (sum_m attn*vb = vb*denominator)
        # into the output projection bias: bout2 = bout + Wout vb.
        be16_2 = persist.tile([P, 2], b16, tag="be16_2")
        nc.scalar.activation(out=be16_2, in_=be2, func=AF.Copy)
        be16 = [be16_2[:, t:t + 1] for t in range(2)]
        # t0 scale on DVE (bf16 SBUF operands: 4x mode), t1 on ACT - parallel
        nc.vector.tensor_scalar(out=wqs[0], in0=wqkv_sb[0],
                                scalar1=albe[0][0], scalar2=None,
                                op0=ALU.mult)
        nc.scalar.activation(out=wqs[1], in_=wqkv_sb[1], func=AF.Copy,
                             scale=albe[1][0])
        for sel in range(3):
            for t in range(2):
                nc.tensor.matmul(
                    qkvb_ps[:, sel:sel + 1],
                    lhsT=wqkv_sb[t][:, sel * P:(sel + 1) * P],
                    rhs=be16[t], start=(t == 0), stop=(t == 1),
                )
        qb = persist.tile([P, 1], f32, tag="qb")
        nc.vector.tensor_copy(qb, qkvb_ps[:, 0:1])
        kb = persist.tile([P, 1], f32, tag="kb")
        nc.vector.tensor_copy(kb, qkvb_ps[:, 1:2])
        vb16 = persist.tile([P, 1], b16, tag="vb16")
        bout2 = [persist.tile([P, 1], f32, tag=f"bo2{t}", name=f"bo2{t}")
                 for t in range(2)]

        def emit_bout2():
            # deferred off the prologue critical path (first needed by the
            # j=0 epilogue, ~85us in)
            nc.vector.tensor_copy(vb16, qkvb_ps[:, 2:3])
            for t in range(2):
                bo_ps = psA.tile([P, 1], f32, tag="sim", name=f"bo_ps{t}")
                nc.tensor.matmul(bo_ps, lhsT=wout_sb[:, t * P:(t + 1) * P],
                                 rhs=vb16, start=True, stop=True)
                nc.vector.tensor_add(bout2[t], bo_ps, bout_sb[t])

        # ---------------- QKV projections ----------------
        qT = persist.tile([P, NQ], b16, tag="qT")
        kT = persist.tile([P, N], b16, tag="kT")
        vS = persist.tile([P, N], b16, tag="vS")   # vS[p, i*128+o] = v[i*128+p, o]

        def emit_q(jq):
            ps = psA.tile([P, 2, JW], f32, tag="sim")
            for t in range(2):
                nc.tensor.matmul(
                    ps[:, 0, :], lhsT=wqs[t][:, 0:P],
                    rhs=xb[t][:, jq * 512:(jq + 1) * 512],
                    start=(t == 0), stop=(t == 1),
                )
            nc.vector.tensor_scalar(out=qT[:, jq * 512:(jq + 1) * 512],
                                    in0=ps[:, 0, :], scalar1=qb,
                                    scalar2=None, op0=ALU.add)

        def emit_k(jk, splits=(512,)):
            base = jk * 512
            lo = 0
            for hi in splits:
                ps = psA.tile([P, 2, JW], f32, tag="sim")
                for t in range(2):
                    nc.tensor.matmul(
                        ps[:, 0, 0:hi - lo], lhsT=wqs[t][:, P:2 * P],
                        rhs=xb[t][:, base + lo:base + hi],
                        start=(t == 0), stop=(t == 1),
                    )
                nc.scalar.activation(out=kT[:, base + lo:base + hi],
                                     in_=ps[:, 0, 0:hi - lo],
                                     func=AF.Identity, bias=kb, scale=1.0)
                lo = hi

        def emit_vS(ch):
            # one 512-token chunk of v, produced DIRECTLY in the attention
            # layout vS[m, o]: the x chunk is the stationary operand, so
            # out = x_chunk^T @ Wv' = v[m, o] -- no transposes needed.
            ps = psA.tile([P, 2, JW], f32, tag="sim", name="vsps")
            for blk in range(4):
                base = ch * 512 + blk * 128
                for t in range(2):
                    nc.tensor.matmul(
                        ps[:, 0, blk * 128:(blk + 1) * 128],
                        lhsT=xb[t][:, base:base + 128],
                        rhs=wqs[t][:, 2 * P:3 * P],
                        start=(t == 0), stop=(t == 1),
                    )
            if ch % 2 == 0:
                nc.scalar.activation(out=vS[:, ch * 512:(ch + 1) * 512],
                                     in_=ps[:, 0, :], func=AF.Copy)
            else:
                nc.vector.tensor_copy(vS[:, ch * 512:(ch + 1) * 512],
                                      ps[:, 0, :])

        # Produce only what attention j=0 needs up front; the rest (q 1-3,
        # k 1-7, v 4-31) is emitted interleaved into j=0's i-loop so the
        # first exp starts early.
        emit_k(0, splits=(128, 512))
        emit_q(0)

        # ---------------- attention ----------------
        # Per-j epilogue is emitted as 5 pieces interleaved into the first
        # iterations of the NEXT j (overlaps its serial chain with compute
        # and keeps the PE warm across the boundary).
        def make_epilogue(j, oacc, dn):
            def p0():
                # every partition of the dn bank already holds its head's
                # denominator (dense all-ones dn stationary), so ln reads
                # the psum bank directly -- no select/broadcast pass.
                lnd = work.tile([P, JW], f32, tag="lnd")
                nc.scalar.activation(out=lnd, in_=dn, func=AF.Ln)
                return lnd

            def p1(lnd):
                rcb = work.tile([P, JW], f32, tag="rcb")
                nc.scalar.activation(out=rcb, in_=lnd, func=AF.Exp, scale=-1.0)
                return rcb

            def p1b(rcb):
                ao = work.tile([P, JW], b16, tag="ao")
                nc.vector.tensor_mul(ao, oacc, rcb)
                return ao

            def p2(ao, t):
                yps = psA.tile([P, JW], f32, tag="sim")
                nc.tensor.matmul(
                    yps, lhsT=wout_sb[:, t * P:(t + 1) * P], rhs=ao,
                    start=True, stop=True,
                )
                ysb = work.tile([P, JW], f32, tag=f"ysb{t}", name=f"ysb{t}")
                # bias add on ACT (Identity with per-partition bias): keeps
                # the busier DVE free for the fexp stream
                nc.scalar.activation(out=ysb, in_=yps, func=AF.Identity,
                                     bias=bout2[t], scale=1.0)
                nc.sync.dma_start(
                    out=y_out[t * P:(t + 1) * P, j * JW:(j + 1) * JW], in_=ysb
                )

            state = {}

            def run_piece(k):
                if k == 0:
                    state["lnd"] = p0()
                elif k == 1:
                    state["rcb"] = p1(state["lnd"])
                elif k == 2:
                    state["ao"] = p1b(state["rcb"])
                elif k == 3:
                    p2(state["ao"], 0)
                elif k == 4:
                    p2(state["ao"], 1)

            return run_piece

        NPIECE = 5
        AVDELAY = 3
        epilogue = None
        pending = []        # av/dn emission pipeline, carried ACROSS j
        for j in range(NJ):
            oacc = psB.tile([P, JW], f32, tag="oacc")
            dn = psB.tile([P, JW], f32, tag="dn")

            def emit_avdn(i, at0, at1, oacc=oacc, dn=dn):
                rhss = [at0[:, 0, :], at0[:, 1, :],
                        at1[:, 0, :].bitcast(b16), at1[:, 1, :].bitcast(b16)]
                for h in range(HEAD):
                    nc.tensor.matmul(
                        oacc[32 * h:32 * h + 32, :],
                        lhsT=vS[:, i * P + 32 * h:i * P + 32 * h + 32],
                        rhs=rhss[h],
                        start=(i == 0), stop=(i == NI - 1),
                        tile_position=(0, 32 * h),
                        skip_group_check=True,
                    )
                for h in range(HEAD):
                    # dense all-ones stationary: all 32 partitions of each
                    # head's dn block receive the denominator (broadcast
                    # done by the PE for free; full tile utilization).
                    nc.tensor.matmul(
                        dn[32 * h:32 * h + 32, :],
                        lhsT=ones32,
                        rhs=rhss[h],
                        start=(i == 0), stop=(i == NI - 1),
                        tile_position=(0, 32 * h),
                        skip_group_check=True,
                    )

            for i in range(NI):
                if j == 0:
                    if i == 0:
                        emit_vS(0)
                    elif (i + 3) % 4 == 0 and i <= 25:
                        ch = (i + 3) // 4
                        emit_k(ch)
                        emit_vS(ch)
                    elif i == 3:
                        emit_bout2()
                    if i in (2, 4, 6):
                        emit_q(i // 2)
                sims = []
                for pr in range(2):
                    sim = psA.tile([P, 2, JW], f32, tag="sim")
                    for hh in range(2):
                        h = pr * 2 + hh
                        nc.tensor.matmul(
                            sim[:, hh, :],
                            lhsT=kT[32 * h:32 * h + 32, i * P:(i + 1) * P],
                            rhs=qT[32 * h:32 * h + 32, j * JW:(j + 1) * JW],
                            start=True, stop=True,
                            tile_position=(32 * h, 0),
                        )
                    sims.append(sim)
                # heads 0,1: exact exp on ACT; heads 2,3: fast-exp on DVE
                at0 = attnp.tile([P, 2, JW], b16, tag="at0", bufs=5)
                nc.scalar.activation(out=at0, in_=sims[0], func=AF.Exp,
                                     scale=SCALE)
                at1 = attnp.tile([P, 2, JW], u16, tag="at1", bufs=5)
                nc.vector.tensor_scalar(
                    out=at1, in0=sims[1], scalar1=FE_A, scalar2=FE_B,
                    op0=ALU.mult, op1=ALU.add,
                )
                pending.append((emit_avdn, i, at0, at1))
                if len(pending) > AVDELAY:
                    fn, ii, a0, a1 = pending.pop(0)
                    fn(ii, a0, a1)
                if epilogue is not None and 2 <= i < 2 + NPIECE:
                    epilogue(i - 2)
                    if i == 1 + NPIECE:
                        epilogue = None
            epilogue = make_epilogue(j, oacc, dn)
        for fn, ii, a0, a1 in pending:
            fn(ii, a0, a1)
        for k in range(NPIECE):
            epilogue(k)

    nc.finalize()
    _cache["nc"] = nc
    return nc


def _prep_in_maps(x, gn_weight, gn_bias, w_qkv, w_out, b_out):
    import ml_dtypes

    f = np.float32
    bf = ml_dtypes.bfloat16
    x = np.asarray(x, dtype=f).astype(bf)
    wqkvT = np.ascontiguousarray(np.asarray(w_qkv, dtype=f).T.astype(bf))
    woutT = np.ascontiguousarray(np.asarray(w_out, dtype=f).T.astype(bf))
    gnw = np.asarray(gn_weight, dtype=f).reshape(2, P)
    gnb = np.asarray(gn_bias, dtype=f).reshape(2, P)
    bo = np.asarray(b_out, dtype=f).reshape(2, P)
    ar = np.arange(P)
    # misc pack: cols 0-1 gnw(t0,t1), 2-3 gnb, 4-5 unused, 6-21 blk8
    misc = np.zeros((P, 22), f)
    misc[:, 0] = gnw[0]
    misc[:, 1] = gnw[1]
    misc[:, 2] = gnb[0]
    misc[:, 3] = gnb[1]
    misc[:, 4] = bo[0]
    misc[:, 5] = bo[1]
    misc[ar, 6 + ar // 8] = 0.125
    blk8T = np.zeros((16, P), f)
    blk8T[ar // 8, ar] = 1.0
    shared = dict(wqkvT=wqkvT, woutT=woutT, misc=misc, blk8T=blk8T)
    in_maps = []
    for core in range(NCORES):
        b, half = divmod(core, 2)
        xb = x[b].reshape(DIM, N)
        if half == 0:
            xp = np.ascontiguousarray(xb)
        else:
            xp = np.ascontiguousarray(
                np.concatenate([xb[:, NQ:], xb[:, :NQ]], axis=1)
            )
        in_maps.append(dict(x=xp, **shared))
    return in_maps


def _get_executor():
    """Build the sharded jitted executor once (compiles the NEFF once).

    Returns (exec_fn, meta): exec_fn takes a list of 8 per-core input dicts
    and returns the list of 8 per-core output dicts.  Mirrors
    concourse.bass2jax.run_bass_via_pjrt's multi-core path but caches the
    jax.jit so repeated calls don't recompile.
    """
    if "exec" in _cache:
        return _cache["exec"]
    import jax
    import concourse.mybir as mybir
    from jax.sharding import Mesh, PartitionSpec
    from jax.experimental.shard_map import shard_map
    from concourse import bass2jax

    bass2jax.install_neuronx_cc_hook()
    nc = _get_nc()

    partition_name = (
        nc.partition_id_tensor.name if nc.partition_id_tensor else None
    )
    in_names, out_names, out_avals, zero_outs = [], [], [], []
    for alloc in nc.m.functions[0].allocations:
        if not isinstance(alloc, mybir.MemoryLocationSet):
            continue
        name = alloc.memorylocations[0].name
        if alloc.kind == "ExternalInput":
            if name != partition_name:
                in_names.append(name)
        elif alloc.kind == "ExternalOutput":
            shape = tuple(alloc.tensor_shape)
            dtype = mybir.dt.np(alloc.dtype)
            out_names.append(name)
            out_avals.append(jax.core.ShapedArray(shape, dtype))
            zero_outs.append(np.zeros(shape, dtype))
    n_params = len(in_names)
    n_outs = len(out_names)
    all_names = in_names + out_names
    if partition_name is not None:
        all_names = all_names + [partition_name]

    def _body(*args):
        operands = list(args)
        if partition_name is not None:
            operands.append(bass2jax.partition_id_tensor())
        outs = bass2jax._bass_exec_p.bind(
            *operands,
            out_avals=tuple(out_avals),
            in_names=tuple(all_names),
            out_names=tuple(out_names),
            lowering_input_output_aliases=(),
            sim_require_finite=True,
            sim_require_nnan=True,
            nc=nc,
        )
        return tuple(outs)

    devices = jax.devices()[:NCORES]
    mesh = Mesh(np.asarray(devices), ("core",))
    sharded = jax.jit(
        shard_map(
            _body, mesh=mesh,
            in_specs=(PartitionSpec("core"),) * (n_params + n_outs),
            out_specs=(PartitionSpec("core"),) * n_outs,
            check_rep=False,
        ),
        keep_unused=True,
    )
    from jax.sharding import NamedSharding
    sharding = NamedSharding(mesh, PartitionSpec("core"))
    dev_zeros = [
        jax.device_put(
            np.zeros((NCORES * z.shape[0], *z.shape[1:]), z.dtype), sharding
        )
        for z in zero_outs
    ]

    def put_inputs(in_maps):
        return [
            jax.device_put(
                np.concatenate([np.asarray(m[name]) for m in in_maps], axis=0),
                sharding,
            )
            for name in in_names
        ]

    def run_device(device_inputs):
        return sharded(*device_inputs, *dev_zeros)

    def exec_fn(in_maps, device_inputs=None):
        if device_inputs is None:
            device_inputs = put_inputs(in_maps)
        out_arrs = [np.asarray(a) for a in run_device(device_inputs)]
        return [
            {
                name: out_arrs[i].reshape(NCORES, *out_avals[i].shape)[c]
                for i, name in enumerate(out_names)
            }
            for c in range(NCORES)
        ]

    meta = dict(in_names=in_names, out_names=out_names, mesh=mesh,
                sharded=sharded, zero_outs=zero_outs,
                put_inputs=put_inputs, run_device=run_device)
    _cache["exec"] = (exec_fn, meta)
    return _cache["exec"]


def _assemble(results):
    y = np.empty((4, DIM, N), np.float32)
    for core in range(NCORES):
        b, half = divmod(core, 2)
        y[b][:, half * NQ:(half + 1) * NQ] = results[core]["y"]
    return y.reshape(4, DIM, 64, 64)


def _run(inputs, **kw):
    exec_fn, _ = _get_executor()
    in_maps = _prep_in_maps(**inputs)
    results = exec_fn(in_maps)
    return _assemble(results), results


def kernel(x, gn_weight, gn_bias, w_qkv, w_out, b_out):
    out, _ = _run(dict(x=x, gn_weight=gn_weight, gn_bias=gn_bias,
                       w_qkv=w_qkv, w_out=w_out, b_out=b_out))
    return out
